# revision 55
# baseline (speedup 1.0000x reference)
# Multi-head self-attention with RoPE on 8 Trainium2 NeuronCores.
#
# Sharding: batch x head-group. Core c handles batch b = c//4 and heads
# 4*(c%4) .. 4*(c%4)+3 (4 of 16 heads), organized as 2 "pr" pairs of 2
# heads. Each core computes Q/K/V projections for its heads from the full
# (transposed) x[b], runs attention, and produces a partial output
# projection Y_partial = O_core^T.T @ Wo[rows-of-its-heads] in bf16. The
# host sums the four partials per batch (in f32) and adds the bias terms.
#
# Everything on-device is bf16 (inputs, Q/K/V, P=exp(S), O, Wo, Y) with
# f32 PSUM accumulation; measured end-to-end rel err ~7e-3 vs the f32
# reference (budget 2e-2). The softmax scale (1/8) is baked into Wq/qb
# host-side so the exp runs with scale=1.0.
#
# Steady state is PE-streaming-bound with the ACT exp stream a close
# second (exp [128,1024] busy ~1.11us; S-pair+PV-pair ~1.0us/slot):
#
#   - flat 128-slot pipeline over units (pr, qt): at slot k the S-pair
#     for slot k+4 is emitted, then PV(k-1), then fillers. The exp PSUM
#     ring is TWO separate [128,1024] tiles (even/odd slot parity): a
#     single [128,2048] tile serializes S(s+1) behind exp(s) via a
#     whole-tile WAR (tile-granularity hazard tracking), capping the
#     cadence at exp+S-pair (~1.66us). Split, S(s+1) only WARs exp(s-1)
#     and runs concurrent with exp(s): cadence ~= exp busy + sem.
#     PSUM: 2 ring tiles (4 banks) + 2 oacc banks + 2 proj/V/WO banks.
#   - input DMAs are spread over the THREE DMA-capable queues (sync/
#     scalar/gpsimd, ~90GB/s each concurrent): each queue's payload is
#     packed contiguously in DRAM ("qs"/"qa"/"qg") and lands in one SBUF
#     mega-tile, split into pieces whose ranges overlap by ONE column -
#     the overlapping write is a real WAW dep that forces the per-queue
#     DMA order (the Tile scheduler otherwise reorders same-queue DMAs
#     arbitrarily, which cost ~10us of startup). x quarter 0 + wq0/wk0 +
#     tbl0 lead their queues so the first exp fires ~23us in (vs ~31
#     with 2-queue staging). The PE is warmed with FULL-SIZE dummy
#     matmuls on a memset tile until the weights land: a cold PE runs
#     matmuls ~1.7x slow, and tiny warm matmuls do NOT hold the p-state.
#   - fillers are emitted in HALF-projection granularity (4 of 8 D-chunks)
#     so a filler never delays a slot's S-pair by more than ~1.1us; K
#     quarters 2-3 go early (S(4q) needs them; K1 and V0-5 live in the
#     prologue), Q quarters are deferred to just before their unit, V
#     tiles 6-15 land 5 slots before their PV, pr1 projections+ropes
#     weave through units 2-3, and the WO output projection is split
#     into eh-halves spread over units 5-7 AFTER each slot's S/PV.
#   - at each unit boundary the [65,512] O accumulators (row 64 = Z via
#     the ones-column-in-V trick) are evicted raw to SBUF with two fast
#     DVE copies; the softmax normalization (reciprocal_approx_fast +
#     gpsimd partition_broadcast + DVE multiply) runs later, off the
#     critical path. NOTE: custom DVE ops (reciprocal_approx_*) need
#     SBUF operands at partition base 0 - PSUM reads or partition-offset
#     inputs silently corrupt on HW.
#   - tail: the last unit's normalize reads its oacc PSUM directly and is
#     chunked per 128-col t-chunk, each chunk immediately releasing its
#     WO sub-projection (casts on the now-idle ACT engine) + output DMA
#     (alternating queues; last two transfers split across both); the
#     z-copies run on ACT, the hh1 broadcast is a PE rank-1 fp32 matmul
#     into the free odd ring tile, and full-size dummy matmuls keep the
#     PE clocked through the normalize gap so WO runs at speed.
#
# RoPE: head-dim rows are pair-interleaved (d' = [0,32,1,33,...]) via a
# host permutation of Wq/Wk columns so the rotate-half partner lives on
# the adjacent partition; a DVE stream_shuffle (pair swap on u32-bitcast
# bf16 pairs) + 2 muls + 1 add apply the rotation per 512-col t-quarter.

import os
import sys

import numpy as np

try:
    import ml_dtypes

    BF16 = np.dtype(ml_dtypes.bfloat16)
except ImportError:  # pragma: no cover
    BF16 = None

for _p in ("/opt/trn_rl_repo", os.path.expanduser("~/.axon_site/_ro/trn_rl_repo")):
    if os.path.isdir(_p) and _p not in sys.path:
        sys.path.insert(0, _p)

B, T, D = 2, 2048, 1024
NHEADS, HD, HALF = 16, 64, 32
HPC = 4  # heads per core
N_CORES = 8
ROPE_BASE = 10000.0
SCALE = float(HD) ** -0.5  # 0.125
NDC = D // 128  # 8 contraction chunks for the projections
NKC = T // 128  # 16 k chunks per head

_SHUF_MASK = [i ^ 1 for i in range(32)]

_ctx: dict = {}


def _build_nc(iters: int = 0, phase: str = "full"):
    import concourse.bacc as bacc
    import concourse.mybir as mybir
    import concourse.tile as tile

    f32 = mybir.dt.float32
    bf16 = mybir.dt.bfloat16
    u32 = mybir.dt.uint32
    i32 = mybir.dt.int32
    Exp = mybir.ActivationFunctionType.Exp
    MUL = mybir.AluOpType.mult

    nc = bacc.Bacc("TRN2", target_bir_lowering=False, debug=False)

    # packed inputs: one contiguous DRAM payload per DMA queue, pieces in
    # stream order (see _host_inputs for layouts)
    qs_d = nc.dram_tensor("qs", [128, 10240], bf16, kind="ExternalInput").ap()
    qa_d = nc.dram_tensor("qa", [128, 9216], bf16, kind="ExternalInput").ap()
    qg_d = nc.dram_tensor("qg", [128, 9216], bf16, kind="ExternalInput").ap()
    bia_d = nc.dram_tensor("bia", [128, 4], f32, kind="ExternalInput").ap()
    y_d = nc.dram_tensor("y", [T, D], bf16, kind="ExternalOutput").ap()
    y_r = y_d.rearrange("(b p) e -> p b e", p=128)  # [128, 16, 1024]

    with tile.TileContext(nc) as tc:
        with (
            tc.tile_pool(name="xpool", bufs=4) as xpool,
            tc.tile_pool(name="wpool", bufs=1) as wpool,
            tc.tile_pool(name="qkpool", bufs=4) as qkpool,
            tc.tile_pool(name="shpool", bufs=2) as shpool,
            tc.tile_pool(name="vpool", bufs=16) as vpool,
            tc.tile_pool(name="ppool", bufs=8) as ppool,
            tc.tile_pool(name="otpool", bufs=2) as otpool,
            tc.tile_pool(name="ypool", bufs=2) as ypool,
            tc.tile_pool(name="rzpool", bufs=4) as rzpool,
            tc.tile_pool(name="rbpool", bufs=4) as rbpool,
            tc.tile_pool(name="pring", bufs=1, space="PSUM") as pring,
            tc.tile_pool(name="pso", bufs=2, space="PSUM") as pso,
            tc.tile_pool(name="psw", bufs=2, space="PSUM") as psw,
        ):

            def body():
                # ---- staged input DMAs across four queues. Per-queue
                # bandwidth ~135GB/s; first-needed pieces lead each queue:
                # x quarter 0 (4 chunk-pair pieces) + wq0/wk0 + tbl q0.
                bia_t = wpool.tile([128, 4], f32, tag="bia", name="bia_t")
                nc.sync.dma_start(out=bia_t[:], in_=bia_d)
                # One SBUF mega-tile per DMA queue; each queue's payload is
                # packed contiguously in DRAM in stream order. Per-queue DMA
                # ORDER IS FORCED by extending each piece's range one column
                # into the next piece: the overlapping write is a real WAW
                # dep, which the scheduler cannot reorder (it freely
                # scrambles same-queue DMAs otherwise, pushing startup-
                # critical pieces several entries back).
                #   qs(sync):  wqk0[0:2048] tbl0[2048:3072] x1h[3072:5120]
                #              x3l[5120:7168] tbl2[7168:8192] wqk1[8192:10240]
                #   qa(scalar):x0l[0:2048] x1l[2048:4096] x2l[4096:6144]
                #              tbl3[6144:7168] x3h[7168:9216]
                #   qg(gpsimd):x0h[0:2048] wv[2048:4096] tbl1[4096:5120]
                #              x2h[5120:7168] wo[7168:9216]
                ms = wpool.tile([128, 10240], bf16, tag="ms", name="ms")
                ma = wpool.tile([128, 9216], bf16, tag="ma", name="ma")
                mg = wpool.tile([128, 9216], bf16, tag="mg", name="mg")
                for eng, dram, mt, cuts in (
                    (nc.sync, qs_d, ms, [0, 2048, 3072, 5120, 7168, 8192, 10240]),
                    (nc.scalar, qa_d, ma, [0, 2048, 4096, 6144, 7168, 9216]),
                    (nc.gpsimd, qg_d, mg, [0, 2048, 4096, 5120, 7168, 9216]),
                ):
                    for k in range(len(cuts) - 1):
                        lo, hi = cuts[k], cuts[k + 1]
                        hi = min(hi + 1, cuts[-1])  # +1 col = WAW chain link
                        eng.dma_start(out=mt[:, lo:hi], in_=dram[:, lo:hi])

                # x chunk-half locations: (quarter, half) -> (tile, base col)
                XMAP = {
                    (0, 0): (ma, 0), (0, 1): (mg, 0),
                    (1, 0): (ma, 2048), (1, 1): (ms, 3072),
                    (2, 0): (ma, 4096), (2, 1): (mg, 5120),
                    (3, 0): (ms, 5120), (3, 1): (ma, 7168),
                }
                TMAP = {0: (ms, 2048), 1: (mg, 4096), 2: (ms, 7168), 3: (ma, 6144)}
                WQKB = [(ms, 0), (ms, 8192)]  # per-pr wq|wk base

                def xsl(q, ch, lo, hi):  # x slice for quarter q, chunk ch
                    mt, base = XMAP[(q, ch // 4)]
                    return mt[:, base + (ch % 4) * 512 + lo : base + (ch % 4) * 512 + hi]

                def tbl_sl(q, lo, hi):
                    mt, base = TMAP[q]
                    return mt[:, base + lo : base + hi]

                def wv_sl(lo, hi):
                    return mg[:, 2048 + lo : 2048 + hi]

                def wo_sl(lo, hi):
                    return mg[:, 7168 + lo : 7168 + hi]

                # two independent ring tiles (even/odd slot parity) - see
                # header comment on the whole-tile WAR this avoids
                rings = [
                    pring.tile([128, 1024], f32, tag=f"ring{p}", name=f"ring{p}")
                    for p in range(2)
                ]

                # ---- Q/K projections in half-granularity (4 D-chunks per
                # emission); eviction = DVE tensor_scalar (+bias) in half 1
                proj_ps = {}

                def project_h(pr, is_k, dst, q, half, evict_eng=None):
                    key = (pr, is_k, q)
                    if half == 0:
                        proj_ps[key] = psw.tile(
                            [128, 512], f32, tag="pw", name=f"ps_{pr}{is_k}{q}"
                        )
                    ps = proj_ps[key]
                    wmt, wbase = WQKB[pr]
                    chs = range(half * 4, half * 4 + 4)
                    for i, ch in enumerate(chs):
                        wcol = wbase + is_k * 1024 + ch * 128
                        nc.tensor.matmul(
                            ps[:],
                            wmt[:, wcol : wcol + 128],
                            xsl(q, ch, 0, 512),
                            start=(half == 0 and i == 0),
                            stop=(half == 1 and i == 3),
                        )
                    if half == 1:
                        dsl = dst[:, q * 512 : (q + 1) * 512]
                        bsl = bia_t[:, is_k * 2 + pr : is_k * 2 + pr + 1]
                        if evict_eng is nc.scalar:
                            # ACT Identity(+per-partition bias): lets the
                            # prologue evictions run off the DVE rope chain
                            nc.scalar.activation(
                                dsl, ps[:],
                                mybir.ActivationFunctionType.Identity,
                                bias=bsl,
                            )
                        else:
                            nc.vector.tensor_scalar_add(dsl, ps[:], bsl)
                        proj_ps.pop(key)

                def rope(t_, name, q):  # rope one 512-col t-quarter in place
                    sl = slice(q * 512, (q + 1) * 512)
                    cos_q = tbl_sl(q, 0, 512)
                    sin_q = tbl_sl(q, 512, 1024)
                    sh = shpool.tile([128, 512], bf16, tag="sh", name=f"sh_{name}{q}")
                    nc.vector.stream_shuffle(
                        sh.bitcast(u32)[:], t_.bitcast(u32)[:, q * 256 : (q + 1) * 256],
                        _SHUF_MASK,
                    )
                    nc.vector.tensor_tensor(
                        out=t_[:, sl], in0=t_[:, sl], in1=cos_q, op=MUL
                    )
                    nc.vector.tensor_tensor(out=sh[:], in0=sh[:], in1=sin_q, op=MUL)
                    nc.vector.tensor_tensor(
                        out=t_[:, sl], in0=t_[:, sl], in1=sh[:], op=mybir.AluOpType.add
                    )

                def rope_chunk(t_, name, q, c):
                    # rope one 128-col chunk of a quarter: lets S(kc) fire as
                    # soon as its kc chunk is roped (startup critical path)
                    cl = slice(q * 512 + c * 128, q * 512 + (c + 1) * 128)
                    cos_c = tbl_sl(q, c * 128, (c + 1) * 128)
                    sin_c = tbl_sl(q, 512 + c * 128, 512 + (c + 1) * 128)
                    sh = shpool.tile(
                        [128, 128], bf16, tag="shc", name=f"shc_{name}{q}_{c}"
                    )
                    nc.vector.stream_shuffle(
                        sh.bitcast(u32)[:],
                        t_.bitcast(u32)[:, q * 256 + c * 64 : q * 256 + (c + 1) * 64],
                        _SHUF_MASK,
                    )
                    nc.vector.tensor_tensor(
                        out=t_[:, cl], in0=t_[:, cl], in1=cos_c, op=MUL
                    )
                    nc.vector.tensor_tensor(out=sh[:], in0=sh[:], in1=sin_c, op=MUL)
                    nc.vector.tensor_tensor(
                        out=t_[:, cl], in0=t_[:, cl], in1=sh[:],
                        op=mybir.AluOpType.add,
                    )

                qts, kts = [], []
                for pr in range(2):
                    qts.append(qkpool.tile([128, T], bf16, tag="qk", name=f"qt{pr}"))
                    kts.append(qkpool.tile([128, T], bf16, tag="qk", name=f"kt{pr}"))

                # ---- V projection halves (bf16 tiles, ones col via memset)
                vts = [None] * NKC
                vps = {}

                def vproj_h(tk, half):
                    if half == 0:
                        vt = vpool.tile([128, HPC * 65], bf16, tag="v", name=f"v{tk}")
                        nc.vector.memset(
                            vt.rearrange("p (h c) -> p h c", c=65)[:, :, 64:65], 1.0
                        )
                        ps = psw.tile([128, 256], f32, tag="pw", name=f"psv{tk}")
                        vps[tk] = (vt, ps)
                    vt, ps = vps[tk]
                    for i in range(4):
                        ch = half * 4 + i
                        nc.tensor.matmul(
                            ps[:],
                            xsl(tk // 4, ch, (tk % 4) * 128, (tk % 4) * 128 + 128),
                            wv_sl(ch * 256, (ch + 1) * 256),
                            start=(ch == 0),
                            stop=(ch == NDC - 1),
                        )
                    if half == 1:
                        nc.vector.tensor_copy(
                            vt.rearrange("p (h c) -> p h c", c=65)[:, :, 0:64],
                            ps.rearrange("p (h c) -> p h c", c=64),
                        )
                        vts[tk] = vt
                        vps.pop(tk)

                ot0 = otpool.tile([128, T], bf16, tag="o", name="ot0")
                ot1 = otpool.tile([128, T], bf16, tag="o", name="ot1")
                ots = [ot0, ot1]

                # ---- flat attention pipeline over 128 global kc-slots ----
                # slot s -> unit u = s//16 = (pr = u//4, qt qi = u%4), kc = s%16.
                # Emission: prologue S(0..3); then slot k: [S(k+4)] [PV(k)]
                # [fillers]; exp(s) is emitted right after S(s).
                NSLOT = 128
                pts = [None] * NSLOT
                oaccs = {}

                def emit_S(s):
                    u, kc = s // 16, s % 16
                    pr, qi = u // 4, u % 4
                    rg = rings[s % 2]
                    for hh in range(2):
                        nc.tensor.matmul(
                            rg[:, hh * 512 : (hh + 1) * 512],
                            kts[pr][hh * 64 : hh * 64 + 64, kc * 128 : (kc + 1) * 128],
                            qts[pr][hh * 64 : hh * 64 + 64, qi * 512 : qi * 512 + 512],
                            start=True,
                            stop=True,
                        )
                    # softmax scale is baked into Wq host-side; plain exp here
                    pt = ppool.tile([128, 1024], bf16, tag="p", name=f"p_{s}")
                    nc.scalar.activation(
                        pt[:], rg[:, 0:1024], Exp, bias=0.0, scale=1.0
                    )
                    pts[s] = pt

                def emit_PV(s):
                    u, kc = s // 16, s % 16
                    pr, qi = u // 4, u % 4
                    if kc == 0:
                        oaccs[u] = [
                            pso.tile([65, 512], f32, tag="oa", name=f"o_{u}_{hh}")
                            for hh in range(2)
                        ]
                    for hh in range(2):
                        nc.tensor.matmul(
                            oaccs[u][hh][:],
                            vts[kc][:, (2 * pr + hh) * 65 : (2 * pr + hh + 1) * 65],
                            pts[s][:, hh * 512 : (hh + 1) * 512],
                            start=(kc == 0),
                            stop=(kc == NKC - 1),
                        )
                    pts[s] = None

                # raw O eviction: one fast DVE copy per hh frees the oacc
                # PSUM bank ~0.4us after the unit's last PV; the normalize
                # chain then runs entirely in SBUF, off the critical path.
                raws = {}

                def raw_copy(u):
                    raw = rzpool.tile([64, 2048], f32, tag="rz", name=f"raw_{u}")
                    # Z rows land in a separate partition-base-0 tile: the
                    # custom DVE reciprocal requires base-0 operands
                    zr = rzpool.tile([1, 1024], f32, tag="zr", name=f"zr_{u}")
                    for hh in range(2):
                        nc.vector.tensor_copy(
                            raw[:, hh * 512 : (hh + 1) * 512], oaccs[u][hh][0:64, :]
                        )
                        nc.vector.tensor_copy(
                            zr[0:1, hh * 512 : (hh + 1) * 512], oaccs[u][hh][64:65, :]
                        )
                    raws[u] = (raw, zr)

                def normalize(u):
                    pr, qi = u // 4, u % 4
                    qs = qi * 512
                    raw, zr = raws[u]
                    for hh in range(2):
                        hs = slice(hh * 512, (hh + 1) * 512)
                        rz = rbpool.tile([1, 512], f32, tag="rz", name=f"rzz_{u}_{hh}")
                        nc.vector.reciprocal_approx_fast(rz[0:1, :], zr[0:1, hs])
                        rc = rbpool.tile([64, 512], f32, tag="rb", name=f"rc_{u}_{hh}")
                        nc.gpsimd.partition_broadcast(
                            rc[:, :], rz[0:1, :], channels=64
                        )
                        nc.vector.tensor_tensor(
                            out=ots[pr][hh * 64 : hh * 64 + 64, qs : qs + 512],
                            in0=raw[0:64, hs],
                            in1=rc[:, :],
                            op=MUL,
                        )

                # wo half (qi, j, eh): t-chunk tt = 4*qi+j, output cols half
                # eh (one matmul pair + cast); qi<3 packs 2 chunks per
                # [128,2048] ysb + 1 DMA, the tail (qi=3) DMAs per chunk.
                ysbs = {}

                def wo_half(qi, j, eh, dma_eng=None, cast_on_act=False, yps_ap=None):
                    tt = 4 * qi + j
                    if j % 2 == 0 and eh == 0:
                        ysbs[(qi, j // 2)] = ypool.tile(
                            [128, 2048], bf16, tag="y", name=f"y_{qi}_{j // 2}"
                        )
                    ysb = ysbs[(qi, j // 2)]
                    if yps_ap is not None:
                        yps = yps_ap
                    else:
                        yps = psw.tile([128, 512], f32, tag="pw", name=f"yp_{tt}_{eh}")
                    for r in range(2):
                        nc.tensor.matmul(
                            yps[:],
                            ots[r][:, tt * 128 : (tt + 1) * 128],
                            wo_sl(r * 1024 + eh * 512, r * 1024 + (eh + 1) * 512),
                            start=(r == 0),
                            stop=(r == 1),
                        )
                    ydst = ysb[
                        :, (j % 2) * 1024 + eh * 512 : (j % 2) * 1024 + (eh + 1) * 512
                    ]
                    if cast_on_act:
                        # ACT is idle after the last exp; keeps DVE off the
                        # tail critical path
                        nc.scalar.activation(
                            ydst, yps[:], mybir.ActivationFunctionType.Copy
                        )
                    else:
                        nc.vector.tensor_copy(ydst, yps[:])
                    if qi == 3:
                        if eh == 1:
                            yv = ysb.rearrange("p (b e) -> p b e", e=1024)
                            if dma_eng == "split":
                                # halve the final transfers across both queues
                                nc.sync.dma_start(
                                    out=y_r[:, tt : tt + 1, 0:512],
                                    in_=yv[:, j % 2 : j % 2 + 1, 0:512],
                                )
                                nc.scalar.dma_start(
                                    out=y_r[:, tt : tt + 1, 512:1024],
                                    in_=yv[:, j % 2 : j % 2 + 1, 512:1024],
                                )
                            else:
                                eng = dma_eng if dma_eng is not None else nc.sync
                                eng.dma_start(
                                    out=y_r[:, tt : tt + 1, :],
                                    in_=yv[:, j % 2 : j % 2 + 1, :],
                                )
                    elif j % 2 == 1 and eh == 1:
                        nc.sync.dma_start(
                            out=y_r[:, 4 * qi + j - 1 : 4 * qi + j + 1, :],
                            in_=ysb.rearrange("p (b e) -> p b e", e=1024),
                        )

                # ---- filler schedule: slot -> list of thunks, emitted AFTER
                # the slot's S/PV so fillers never delay the exp stream; all
                # placements are deadline-driven (producer finishes >=2 slots
                # before its consumer's S/PV).
                fillers = {}

                def add_filler(slot, fn):
                    fillers.setdefault(slot, []).append(fn)

                def qk_h(q, is_k, half):
                    def fn():
                        project_h(0, is_k, kts[0] if is_k else qts[0], q, half)

                    return fn

                def qk_rope(q, is_k):
                    def fn():
                        rope(
                            kts[0] if is_k else qts[0],
                            "k0" if is_k else "q0",
                            q,
                        )

                    return fn

                # K quarters 2-3 early (S(4q) is EMITTED at slot 4q-4, so
                # quarter q's emission must complete by then; K1 lives in
                # the prologue). Q quarter q deferred to just before unit q.
                for q, s0 in ((2, 2), (3, 6)):
                    add_filler(s0, qk_h(q, 1, 0))
                    add_filler(s0 + 1, qk_h(q, 1, 1))
                    add_filler(s0 + 2, qk_rope(q, 1))
                for q, s0 in ((1, 9), (2, 19), (3, 25)):
                    add_filler(s0, qk_h(q, 0, 0))
                    add_filler(s0 + 1, qk_h(q, 0, 1))
                    add_filler(s0 + 2, qk_rope(q, 0))
                # V tiles 6..15: halves at slots k-5 / k-4 (PV(k) at slot k+1;
                # V0-5 live in the prologue)
                for tk in range(6, NKC):
                    add_filler(tk - 5, (lambda t: lambda: vproj_h(t, 0))(tk))
                    add_filler(tk - 4, (lambda t: lambda: vproj_h(t, 1))(tk))
                # pr1 projections + ropes woven through units 2-3 at 3-slot
                # stride (all needed by S(64), emitted at slot 60)
                pj = 0
                for is_k in range(2):
                    for q in range(4):
                        add_filler(
                            32 + 3 * pj,
                            (lambda ik, qq: lambda: project_h(
                                1, ik, kts[1] if ik else qts[1], qq, 0
                            ))(is_k, q),
                        )
                        add_filler(
                            33 + 3 * pj,
                            (lambda ik, qq: lambda: project_h(
                                1, ik, kts[1] if ik else qts[1], qq, 1
                            ))(is_k, q),
                        )
                        add_filler(
                            34 + 3 * pj,
                            (lambda ik, qq: lambda: rope(
                                kts[1] if ik else qts[1], "k1" if ik else "q1", qq
                            ))(is_k, q),
                        )
                        pj += 1
                # wo halves spread over units 5-7, starting at slot 5: the
                # unit's first slots overlap the previous unit's normalize
                # chain (recip->gpsimd broadcast->muls, ~3 slots deep), and
                # WO needs its freshly normalized ots - scheduling WO there
                # stalled the exp stream ~2.2us mid-unit
                WOSL = [5, 6, 7, 8, 9, 10, 12, 14]
                # qi=2 (unit 7) pulls its comb forward so its last halves
                # don't interleave with the final PVs / tail normalize
                WOSL2 = [4, 5, 6, 7, 8, 9, 10, 11]
                postfill = {}
                for qi in range(3):
                    for j in range(4):
                        for eh in range(2):
                            sl = (WOSL2 if qi == 2 else WOSL)[2 * j + eh]
                            postfill.setdefault((5 + qi) * 16 + sl, []).append(
                                (lambda a, b, c: lambda: wo_half(a, b, c))(qi, j, eh)
                            )

                # ---- prologue: warm the PE p-state on the (tiny, landed)
                # bias tile until wq0/x0 stream in, then quarter 0 ----
                # Warm the PE with FULL-SIZE dummy matmuls on a memset tile:
                # tiny (4x4) warms keep the engine busy but do NOT hold the
                # p-state up — subsequent real matmuls still run ~1.7x slow.
                warmsrc = wpool.tile([128, 512], bf16, tag="wsrc", name="warmsrc")
                nc.vector.memset(warmsrc[:], 1.0)
                ones_f = wpool.tile([1, 64], f32, tag="onef", name="ones_f")
                nc.vector.memset(ones_f[:], 1.0)
                warm = psw.tile([128, 512], f32, tag="pw", name="warm")
                for w in range(18):
                    nc.tensor.matmul(
                        warm[:],
                        warmsrc[:, 0:128],
                        warmsrc[:],
                        start=True,
                        stop=True,
                    )
                # Q/K quarter 0 interleaved at half-granularity (K finishes
                # ~1us after Q); S(0..3) emitted together after the K rope so
                # they are all ready at once (keeps the scheduler from
                # wedging V work between them)
                project_h(0, 0, qts[0], 0, 0)
                project_h(0, 1, kts[0], 0, 0)
                project_h(0, 0, qts[0], 0, 1, evict_eng=nc.scalar)
                project_h(0, 1, kts[0], 0, 1, evict_eng=nc.scalar)
                rope(qts[0], "q0", 0)
                # K0 rope in halves: S(0)/S(1) need only kc 0-1 (cols 0:256)
                rope_chunk(kts[0], "k0h", 0, 0)
                rope_chunk(kts[0], "k0h", 0, 1)
                emit_S(0)
                emit_S(1)
                rope_chunk(kts[0], "k0h", 0, 2)
                rope_chunk(kts[0], "k0h", 0, 3)
                emit_S(2)
                emit_S(3)
                # bridge the wv/x1 DMA wait (~18.4-20us) with full-size
                # warms so the V/K1 projections start at full clock; 4 only,
                # so a fast DMA is never delayed by queued warms
                for w in range(4):
                    nc.tensor.matmul(
                        warm[:],
                        warmsrc[:, 0:128],
                        warmsrc[:],
                        start=True,
                        stop=True,
                    )
                # V0/V1 ahead of K1 (PV(0) is consumed at slot 1); K quarter
                # 1 next (S(4) is emitted at slot 0), then V2..V5
                vproj_h(0, 0)
                vproj_h(0, 1)
                vproj_h(1, 0)
                vproj_h(1, 1)
                project_h(0, 1, kts[0], 1, 0)
                project_h(0, 1, kts[0], 1, 1)
                rope(kts[0], "k0", 1)
                for tk in range(2, 6):
                    vproj_h(tk, 0)
                    vproj_h(tk, 1)

                # ---- steady state; PV lags S-emission by 5 slots ----
                for k in range(NSLOT + 1):
                    if k > 16 and k % 16 == 1:
                        normalize(k // 16 - 1)
                    for fn in fillers.get(k, ()):
                        fn()
                    if k + 4 < NSLOT:
                        emit_S(k + 4)
                    if k > 0:
                        emit_PV(k - 1)
                    if k > 0 and k % 16 == 0 and k // 16 - 1 != 7:
                        raw_copy(k // 16 - 1)
                    for fn in postfill.get(k, ()):
                        fn()

                # ---- tail: last unit's normalize reads the oacc PSUM
                # directly, chunked per t-chunk; each chunk releases its WO
                # half-pair + DMA. Dummy matmuls keep PE clocked through the
                # normalize gap so the WO matmuls run at full speed.
                twarm = psw.tile([128, 512], f32, tag="pw", name="twarm")
                for w in range(8):
                    nc.tensor.matmul(
                        twarm[:],
                        warmsrc[:, 0:128],
                        warmsrc[:],
                        start=True,
                        stop=True,
                    )
                # per-hh normalize chain spread over three engines: z-copy on
                # ACT (idle after last exp), recip on DVE, hh0 broadcast on
                # gpsimd, hh1 broadcast as a PE fp32 rank-1 matmul into the
                # (free) odd ring tile — runs concurrently with the gpsimd
                # one and keeps the PE clock up for the WO matmuls
                # hh1's O is raw-copied to SBUF (DVE is free right after the
                # last PV) so its normalize mul can read SBUF x ring-PSUM
                # (DVE allows only one PSUM operand)
                raw1 = rzpool.tile([64, 512], f32, tag="rz", name="raw_t1")
                nc.vector.tensor_copy(raw1[:, :], oaccs[7][1][0:64, :])
                rbs = []
                for hh in range(2):
                    rz = rzpool.tile([1, 1024], f32, tag="zr", name=f"rzt_{hh}")
                    nc.scalar.activation(
                        rz[0:1, 0:512],
                        oaccs[7][hh][64:65, :],
                        mybir.ActivationFunctionType.Copy,
                    )
                    nc.vector.reciprocal_approx_fast(
                        rz[0:1, 512:1024], rz[0:1, 0:512]
                    )
                    if hh == 0:
                        rb = rbpool.tile([64, 512], f32, tag="rb", name="rbt_0")
                        nc.gpsimd.partition_broadcast(
                            rb[:, :], rz[0:1, 512:1024], channels=64
                        )
                        rbs.append(rb)
                    else:
                        nc.tensor.matmul(
                            rings[1][0:64, 0:512],
                            ones_f[0:1, :],
                            rz[0:1, 512:1024],
                            start=True,
                            stop=True,
                        )
                        rbs.append(rings[1])
                for j in range(4):
                    cs = slice(j * 128, (j + 1) * 128)
                    for hh in range(2):
                        nc.vector.tensor_tensor(
                            out=ots[1][
                                hh * 64 : hh * 64 + 64,
                                3 * 512 + j * 128 : 3 * 512 + (j + 1) * 128,
                            ],
                            in0=oaccs[7][hh][0:64, cs] if hh == 0 else raw1[:, cs],
                            in1=rbs[hh][0:64, cs],
                            op=MUL,
                        )
                    # casts split across ACT (eh0) and DVE (eh1): 8 serial
                    # ACT casts (~5.5us) otherwise gate the WO pipeline.
                    # j2/j3 accumulate in the now-free ring banks so the WO
                    # pairs aren't paced by psw-buffer (cast) recycling
                    # (rings[1][0:64, 0:512] still holds the hh1 broadcast)
                    eng = (nc.sync if j == 0 else nc.scalar) if j < 2 else "split"
                    extra = {
                        (2, 0): rings[0][:, 0:512],
                        (2, 1): rings[0][:, 512:1024],
                        (3, 0): rings[1][:, 512:1024],
                    }
                    wo_half(3, j, 0, dma_eng=eng, cast_on_act=True,
                            yps_ap=extra.get((j, 0)))
                    wo_half(3, j, 1, dma_eng=eng, cast_on_act=False,
                            yps_ap=extra.get((j, 1)))

            if iters:
                import concourse.mybir as _mb

                with tc.For_i(
                    0,
                    iters,
                    1,
                    hint_engines=(
                        _mb.EngineType.PE,
                        _mb.EngineType.Activation,
                        _mb.EngineType.DVE,
                        _mb.EngineType.SP,
                        _mb.EngineType.Pool,
                    ),
                    staggered_reset=True,
                ) as _iv:
                    body()
            else:
                body()

    nc.compile()
    return nc


def _host_inputs(x, wq_w, wq_b, wk_w, wk_b, wv_w, wv_b, wo_w, wo_b):
    """Build the 8 per-core input maps (all host-side slicing/packing)."""
    f = np.float32
    x = np.asarray(x, f)
    wq_w = np.asarray(wq_w, f)
    wk_w = np.asarray(wk_w, f)
    wv_w = np.asarray(wv_w, f)
    wo_w = np.asarray(wo_w, f)
    wq_b = np.asarray(wq_b, f)
    wk_b = np.asarray(wk_b, f)
    wv_b = np.asarray(wv_b, f)
    wo_b = np.asarray(wo_b, f)

    def chunkpack(a, ncol):  # [1024, ncol] -> [128, 8*ncol] (D-chunk packed)
        return np.ascontiguousarray(
            a.reshape(NDC, 128, ncol).transpose(1, 0, 2).reshape(128, NDC * ncol)
        )

    # RoPE tables in fp32, mirroring the reference formulas; stored bf16.
    pos = np.arange(T, dtype=f)[:, None]
    idx = np.arange(HALF, dtype=f)[None, :]
    inv_freq = (f(1.0) / (f(ROPE_BASE) ** (idx / f(HALF)))).astype(f)
    ang = pos * inv_freq  # [T, 32]
    cosv, sinv = np.cos(ang).astype(f), np.sin(ang).astype(f)
    cos64 = np.repeat(cosv.T, 2, axis=0)  # [64, T]
    sin64 = np.repeat(sinv.T, 2, axis=0)
    sin64[0::2] *= -1  # rows 2j: -sin, rows 2j+1: +sin
    cos128 = np.tile(cos64, (2, 1))
    sin128 = np.tile(sin64, (2, 1))
    # quarter-interleaved: [cos_q | sin_q] per 512-col t-quarter
    tbl = np.ascontiguousarray(
        np.concatenate(
            [
                np.concatenate(
                    [cos128[:, q * 512 : (q + 1) * 512], sin128[:, q * 512 : (q + 1) * 512]],
                    axis=1,
                )
                for q in range(4)
            ],
            axis=1,
        ).astype(BF16)
    )

    perm64 = np.empty(64, np.int64)
    perm64[0::2] = np.arange(32)
    perm64[1::2] = np.arange(32) + 32

    # x[b]^T quarter-packed: quarter q holds all 8 D-chunks for t in
    # [512q, 512(q+1)): [128, 8ch x 512t]
    xqp = []
    for b in range(B):
        xt = x[b].T.reshape(NDC, 128, 4, 512)  # [ch, p, q, t]
        xqp.append(
            np.ascontiguousarray(
                xt.transpose(2, 1, 0, 3).reshape(4, 128, NDC * 512)
                .transpose(1, 0, 2).reshape(128, 4 * NDC * 512)
            ).astype(BF16)
        )

    in_maps = []
    for c in range(N_CORES):
        b, g = c // 4, c % 4
        heads = np.arange(4 * g, 4 * g + 4)
        v_cols = np.concatenate([np.arange(h * 64, (h + 1) * 64) for h in heads])
        # softmax scale baked into Wq/qb so the exp runs with scale=1.0
        wqk_parts, bia_cols = [], []
        for w_, b_, sc in ((wq_w, wq_b, np.float32(SCALE)), (wk_w, wk_b, np.float32(1.0))):
            for pr in range(2):
                prheads = heads[2 * pr : 2 * pr + 2]
                cols = np.concatenate([h * 64 + perm64 for h in prheads])
                wqk_parts.append((pr, chunkpack(w_[:, cols] * sc, 128)))
                bia_cols.append((pr, b_[cols] * sc))
        # layout: wq0 | wk0 | wq1 | wk1  (each [128, 1024])
        order = [0, 2, 1, 3]  # indices into wqk_parts (built q0,q1,k0,k1)
        wqk = np.concatenate([wqk_parts[i][1] for i in order], axis=1).astype(BF16)
        # bias cols: qb0, qb1, kb0, kb1
        bia = np.stack(
            [bia_cols[0][1], bia_cols[1][1], bia_cols[2][1], bia_cols[3][1]], axis=1
        ).astype(f)
        wvp = chunkpack(wv_w[:, v_cols], 256).astype(BF16)
        wop = np.ascontiguousarray(
            wo_w[v_cols, :]
            .reshape(2, 128, D)
            .transpose(1, 0, 2)
            .reshape(128, 2 * D)
            .astype(BF16)
        )
        # per-DMA-queue contiguous payloads, pieces in stream order
        # (must mirror the cuts in _build_nc)
        xq = xqp[b]
        qs = np.concatenate(
            [wqk[:, 0:2048], tbl[:, 0:1024], xq[:, 6144:8192],
             xq[:, 12288:14336], tbl[:, 2048:3072], wqk[:, 2048:4096]],
            axis=1,
        )
        qa = np.concatenate(
            [xq[:, 0:2048], xq[:, 4096:6144], xq[:, 8192:10240],
             tbl[:, 3072:4096], xq[:, 14336:16384]],
            axis=1,
        )
        qg = np.concatenate(
            [xq[:, 2048:4096], wvp, tbl[:, 1024:2048],
             xq[:, 10240:12288], wop],
            axis=1,
        )
        in_maps.append(
            {
                "qs": np.ascontiguousarray(qs),
                "qa": np.ascontiguousarray(qa),
                "qg": np.ascontiguousarray(qg),
                "bia": np.ascontiguousarray(bia),
            }
        )

    beff = (
        wo_b.astype(np.float64) + wv_b.astype(np.float64) @ wo_w.astype(np.float64)
    ).astype(f)
    return in_maps, beff


def kernel(x, wq_w, wq_b, wk_w, wk_b, wv_w, wv_b, wo_w, wo_b):
    from concourse import bass2jax

    in_maps, beff = _host_inputs(
        x, wq_w, wq_b, wk_w, wk_b, wv_w, wv_b, wo_w, wo_b
    )
    if "nc" not in _ctx:
        _ctx["nc"] = _build_nc(0)
    res = bass2jax.run_bass_via_pjrt(_ctx["nc"], in_maps, n_cores=N_CORES)
    y = np.empty((B, T, D), np.float32)
    for b in range(B):
        acc = np.asarray(res[4 * b]["y"], np.float32)
        for g in range(1, 4):
            acc += np.asarray(res[4 * b + g]["y"], np.float32)
        y[b] = acc + beff[None, :]
    return y
